# revision 6
# baseline (speedup 1.0000x reference)
"""Trainium2 Bass kernel for nn_Centerdist (segment variance loss).

Math: for each id k in [0, 1000):
    loss_k = sum_{i: id_i=k} ||x_i - mean_k||^2 / n_k
           = (sumsq_k - ||sums_k||^2 / n_k) / n_k
    loss = sum_k loss_k / n_uniq

Sharding strategy: rows are partitioned by id range — core c owns ids
[125c, 125(c+1)).  That makes each core's id window at most 125 wide, so
per-id aggregation needs a single 128-wide one-hot matmul per 128-row
tile (instead of eight 128-wide chunks covering the whole [0,1024) range
when rows are dealt round-robin).  Per tile the device:

  - squares x into the second half of the rhs tile (ACT, with a fraction
    on DVE to balance engine load),
  - builds the one-hot [128 rows, 128 window-ids] via is_equal against a
    window-relative iota (DVE),
  - one matmul accumulates one_hot.T @ [x | x^2] into a persistent
    [128, 512] PSUM bank across all tiles.

The [window, 256] per-(id,d) sums and per-(id,d) sums-of-squares come
back per core; the host reduces the squares over d, combines the eight
disjoint windows, and applies the final per-id division and mean.
Inputs are pre-cast to fp16 on the host so the loads are plain HWDGE
DMAs (the SWDGE cast-DMA path is several times slower).
"""

import numpy as np

from concourse import bacc, bass, bass_utils, mybir, tile

F32 = mybir.dt.float32
F16 = mybir.dt.float16

N_FULL = 262144
D = 256
NUM_IDS = 1000
P = 128
N_CORES = 8
IDS_PER_CORE = (NUM_IDS + N_CORES - 1) // N_CORES  # 125
RW = 2 * D  # rhs width: [x | x^2]
LOAD_T = 8  # tiles per DMA load (512 KB per dma_start)
DVE_SQ_EVERY = 4  # every 4th tile's square computed on DVE instead of ACT


def build_program(tiles: int, reps: int = 1):
    """Build the per-core Bass program processing `tiles` 128-row tiles.

    reps>1 repeats the whole pass (for slope-based HW timing); the output
    is identical since each rep restarts the PSUM accumulation group.
    """
    nc = bacc.Bacc(
        "TRN2",
        target_bir_lowering=False,
        debug=False,
        num_devices=N_CORES,
    )
    ns = tiles * P
    x_d = nc.dram_tensor("x", [ns, D], F16, kind="ExternalInput")
    idst_d = nc.dram_tensor("idst", [P, tiles], F32, kind="ExternalInput")
    iota_d = nc.dram_tensor("iota", [P, P], F16, kind="ExternalInput")
    out_d = nc.dram_tensor("out", [P, RW], F32, kind="ExternalOutput")

    with tile.TileContext(nc) as tc:
        with (
            tc.tile_pool(name="const", bufs=1) as cpool,
            tc.tile_pool(name="xp", bufs=3) as xpool,
            tc.tile_pool(name="ohp", bufs=4) as ohpool,
            tc.tile_pool(name="psp", bufs=1, space="PSUM") as pspool,
            tc.tile_pool(name="evp", bufs=1) as evpool,
        ):
            iota_t = cpool.tile([P, P], F16, tag="iota")
            nc.sync.dma_start(iota_t[:], iota_d.ap())
            idst_t = cpool.tile([P, tiles], F32, tag="idst")
            nc.sync.dma_start(idst_t[:], idst_d.ap())

            psum = pspool.tile([P, 512], F32, name="ps", tag="ps")

            load_t = min(LOAD_T, tiles)
            assert tiles % load_t == 0
            # [group, p, t, d] view of row-major x for multi-tile loads
            x_g = x_d.ap().rearrange("(g t p) d -> g p t d", p=P, t=load_t)

            for rep in range(reps):
                for gi in range(tiles // load_t):
                    xt = xpool.tile([P, load_t, RW], F16, name="xt", tag="xt")
                    nc.sync.dma_start(xt[:, :, 0:D], x_g[gi])

                    for tt in range(load_t):
                        t = gi * load_t + tt
                        # x^2 into the rhs second half
                        if tt % DVE_SQ_EVERY == DVE_SQ_EVERY - 1:
                            nc.vector.tensor_tensor(
                                out=xt[:, tt, D:RW],
                                in0=xt[:, tt, 0:D],
                                in1=xt[:, tt, 0:D],
                                op=mybir.AluOpType.mult,
                            )
                        else:
                            nc.scalar.activation(
                                xt[:, tt, D:RW],
                                xt[:, tt, 0:D],
                                mybir.ActivationFunctionType.Square,
                            )

                        oh = ohpool.tile([P, P], F16, name="oh", tag="oh")
                        nc.vector.tensor_scalar(
                            out=oh[:],
                            in0=iota_t[:],
                            scalar1=idst_t[:, t : t + 1],
                            scalar2=None,
                            op0=mybir.AluOpType.is_equal,
                        )

                        nc.tensor.matmul(
                            psum[:],
                            oh[:],
                            xt[:, tt, :],
                            start=(t == 0),
                            stop=(t == tiles - 1),
                        )

            ev = evpool.tile([P, RW], F32, name="ev", tag="ev")
            nc.vector.tensor_copy(ev[:], psum[:])
            nc.sync.dma_start(out_d.ap(), ev[:])

    nc.compile()
    return nc


_PROGRAM_CACHE: dict = {}


def _get_program(tiles: int, reps: int = 1):
    key = (tiles, reps)
    if key not in _PROGRAM_CACHE:
        _PROGRAM_CACHE[key] = build_program(tiles, reps)
    return _PROGRAM_CACHE[key]


def make_in_maps(reid_feat: np.ndarray, ids: np.ndarray):
    """Partition rows by id range across the 8 cores.

    Core c gets all rows whose id is in [125c, 125(c+1)), padded with
    rel-id -1 rows (which match nothing) to a common tile count.
    """
    x = np.asarray(reid_feat, dtype=np.float32)
    ids_np = np.asarray(ids).astype(np.int64)
    valid = ids_np >= 0

    order = np.argsort(ids_np, kind="stable")
    ids_sorted = ids_np[order]
    # drop invalid (negative) ids — they contribute nothing to the sums
    lo_valid = np.searchsorted(ids_sorted, 0, side="left")
    bounds = np.searchsorted(
        ids_sorted, np.arange(0, NUM_IDS + IDS_PER_CORE, IDS_PER_CORE), side="left"
    )
    bounds[0] = lo_valid
    counts_per_core = np.diff(bounds)
    max_rows = int(counts_per_core.max())
    tiles = max(1, (max_rows + P - 1) // P)
    if tiles > LOAD_T:
        tiles = ((tiles + LOAD_T - 1) // LOAD_T) * LOAD_T  # whole DMA groups
    ns = tiles * P

    iota = np.broadcast_to(
        np.arange(P, dtype=np.float16), (P, P)
    ).copy()  # iota[p, j] = j

    in_maps = []
    for c in range(N_CORES):
        sel = order[bounds[c] : bounds[c + 1]]
        n_c = sel.shape[0]
        xs = np.zeros((ns, D), dtype=np.float16)
        xs[:n_c] = x[sel]
        rel = np.full((ns,), -1.0, dtype=np.float32)
        rel[:n_c] = (ids_sorted[bounds[c] : bounds[c + 1]] - IDS_PER_CORE * c).astype(
            np.float32
        )
        idst = rel.reshape(tiles, P).T.copy()
        in_maps.append({"x": xs, "idst": idst, "iota": iota})
    return in_maps, tiles, valid


def finalize(parts: np.ndarray, ids: np.ndarray, valid: np.ndarray) -> np.ndarray:
    """Combine per-core window partials [cores, 128, 512] into the loss."""
    parts = parts.astype(np.float64)
    K_PAD = IDS_PER_CORE * N_CORES + P
    sums = np.zeros((K_PAD, D))
    sumsq = np.zeros((K_PAD,))
    for c in range(N_CORES):
        base = IDS_PER_CORE * c
        sums[base : base + P] += parts[c, :, 0:D]
        sumsq[base : base + P] += parts[c, :, D:RW].sum(axis=1)
    sums = sums[:NUM_IDS]
    sumsq = sumsq[:NUM_IDS]

    ids_np = np.asarray(ids).astype(np.int64)
    counts = np.bincount(ids_np[valid], minlength=NUM_IDS)[:NUM_IDS].astype(np.float64)
    safe_n = np.maximum(counts, 1.0)
    sq_per_id = sumsq - (sums * sums).sum(axis=1) / safe_n
    per_id_loss = np.where(counts > 0, sq_per_id / safe_n, 0.0)
    n_uniq = float((counts > 0).sum()) + (1.0 if (~valid).any() else 0.0)
    return np.array(per_id_loss.sum() / n_uniq, dtype=np.float32)


def run_device(reid_feat, ids, trace: bool = False):
    in_maps, tiles, valid = make_in_maps(reid_feat, ids)
    nc = _get_program(tiles)
    res = bass_utils.run_bass_kernel_spmd(
        nc, in_maps, core_ids=list(range(N_CORES)), trace=trace
    )
    parts = np.stack([res.results[c]["out"] for c in range(N_CORES)])
    return parts, valid, res


class DeviceRunner:
    """Persistent jitted SPMD executor (mirrors bass2jax.run_bass_via_pjrt)
    so a program can be executed many times for timing without re-tracing."""

    def __init__(self, nc, in_maps, chain: int = 1):
        import jax
        from jax.sharding import Mesh, PartitionSpec
        from jax.experimental.shard_map import shard_map
        from concourse import bass2jax, mybir as mb

        bass2jax.install_neuronx_cc_hook()
        partition_name = (
            nc.partition_id_tensor.name if nc.partition_id_tensor else None
        )
        in_names, out_names, out_avals, zero_outs = [], [], [], []
        for alloc in nc.m.functions[0].allocations:
            if not isinstance(alloc, mb.MemoryLocationSet):
                continue
            name = alloc.memorylocations[0].name
            if alloc.kind == "ExternalInput":
                if name != partition_name:
                    in_names.append(name)
            elif alloc.kind == "ExternalOutput":
                shape = tuple(alloc.tensor_shape)
                npdt = np.dtype(mb.dt.np(alloc.dtype))
                out_names.append(name)
                out_avals.append(jax.core.ShapedArray(shape, npdt))
                zero_outs.append(np.zeros(shape, npdt))
        self.out_names = out_names
        n_params = len(in_names)
        n_outs = len(out_avals)
        all_names = list(in_names) + list(out_names)
        if partition_name is not None:
            all_names.append(partition_name)

        def _body(*args):
            ins = list(args[:n_params])
            outs = list(args[n_params:])
            # chain>1 = several dependent NEFF executions per dispatch, so
            # per-dispatch overhead can be sloped away when timing
            for _ in range(chain):
                operands = ins + outs
                if partition_name is not None:
                    operands.append(bass2jax.partition_id_tensor())
                outs = list(
                    bass2jax._bass_exec_p.bind(
                        *operands,
                        out_avals=tuple(out_avals),
                        in_names=tuple(all_names),
                        out_names=tuple(out_names),
                        lowering_input_output_aliases=(),
                        sim_require_finite=True,
                        sim_require_nnan=True,
                        nc=nc,
                    )
                )
            return tuple(outs)

        devices = jax.devices()[:N_CORES]
        mesh = Mesh(np.asarray(devices), ("core",))
        in_specs = (PartitionSpec("core"),) * (n_params + n_outs)
        out_specs = (PartitionSpec("core"),) * n_outs
        self._fn = jax.jit(
            shard_map(
                _body,
                mesh=mesh,
                in_specs=in_specs,
                out_specs=out_specs,
                check_rep=False,
            ),
            keep_unused=True,
        )
        self._jax = jax
        concat_in = [
            np.concatenate([np.asarray(in_maps[c][nm]) for c in range(N_CORES)], axis=0)
            for nm in in_names
        ]
        concat_zeros = [
            np.zeros((N_CORES * z.shape[0], *z.shape[1:]), z.dtype) for z in zero_outs
        ]
        sharding = jax.sharding.NamedSharding(mesh, PartitionSpec("core"))
        self._args = [jax.device_put(a, sharding) for a in concat_in + concat_zeros]
        self.out_shapes = [a.shape for a in out_avals]

    def run_once(self):
        outs = self._fn(*self._args)
        self._jax.block_until_ready(outs)
        return outs

    def results(self):
        outs = self.run_once()
        return [
            {
                nm: np.asarray(outs[i]).reshape(N_CORES, *self.out_shapes[i])[c]
                for i, nm in enumerate(self.out_names)
            }
            for c in range(N_CORES)
        ]

    def time_exec(self, iters: int = 20, warmup: int = 3):
        import time as _time

        for _ in range(warmup):
            self.run_once()
        times = []
        for _ in range(iters):
            t0 = _time.perf_counter()
            self.run_once()
            times.append(_time.perf_counter() - t0)
        return float(np.median(times)), times


def kernel(reid_feat, ids) -> np.ndarray:
    parts, valid, _ = run_device(reid_feat, ids)
    return finalize(parts, np.asarray(ids), valid)
